# revision 5
# baseline (speedup 1.0000x reference)
"""Distance-aware comb-pilot interpolator for Trainium2 (8 NeuronCores).

Math: out[b, i, c] = (w_l[i] * H[b, j0(i), c] + w_r[i] * H[b, j1(i), c]) / w[i]
with pilots on the comb loc[k] = 8k (k = 0..511), Nfft = 4096.
For i = 8k + r (k < 511): j0 = k, j1 = k + 1 and the normalized weights
depend only on r:  alpha[r] = w_l/w, gamma[r] = w_r/w.
For the last 8 subcarriers (i = 4088..4095) the reference extrapolates a
virtual pilot hN = (15/8)H[511] - (7/8)H[510] at subcarrier 4095; folding it
in gives per-r coefficients on H[510] and H[511] directly.

All coefficients depend only on decay = softplus(decay_param) and are O(8)
host work; they ship to the device as one tiny [128, 48] constant tile.

Device kernel (per core, batch-sharded 512 rows): partition dim = batch.
Per 128-batch tile: ScalarE computes tmp = gamma[r] * H[:, k+1, :], then the
DVE fused op scalar_tensor_tensor writes out[:, k, r, :] =
(H[:, k, :] * alpha[r]) + tmp for all k in one pass.  The kernel moves
2.1 MB in / 16.8 MB out per core and is HBM-bound (~53 us roofline).
"""

import sys

import numpy as np

for _p in ("/opt/trn_rl_repo", "/root/.axon_site/_ro/trn_rl_repo"):
    if _p not in sys.path:
        sys.path.append(_p)

import concourse.bass as bass
import concourse.tile as tile
from concourse import bacc, mybir
from concourse.bass_utils import run_bass_kernel_spmd

N_CORES = 8
B, NP, NFFT, SPACING = 4096, 512, 4096, 8
B_LOC = B // N_CORES  # batch rows per core
NSEG = NP - 1  # regular 8-wide segments (k = 0..510)
P = 128  # SBUF partitions
N_BT = B_LOC // P  # 128-batch tiles per core

_PROGRAM = None


def _build_program():
    """One Bass program, identical on all cores (pure data parallel)."""
    nc = bacc.Bacc("TRN2", target_bir_lowering=False, debug=False)
    f32 = mybir.dt.float32
    ls = nc.dram_tensor("ls", [B_LOC, NP * 2], f32, kind="ExternalInput").ap()
    coef = nc.dram_tensor("coef", [P, 48], f32, kind="ExternalInput").ap()
    out = nc.dram_tensor("out", [B_LOC, NFFT * 2], f32, kind="ExternalOutput").ap()

    mult, add = mybir.AluOpType.mult, mybir.AluOpType.add

    with tile.TileContext(nc) as tc:
        with (
            tc.tile_pool(name="cpool", bufs=1) as cpool,
            tc.tile_pool(name="hpool", bufs=4) as hpool,
            tc.tile_pool(name="opool", bufs=3) as opool,
            tc.tile_pool(name="tpool", bufs=6) as tpool,
            tc.tile_pool(name="lpool", bufs=2) as lpool,
        ):
            ct = cpool.tile([P, 48], f32)
            nc.gpsimd.dma_start(ct[:], coef)

            # Preload every input tile before any output traffic exists —
            # loads issued mid-kernel crawl at ~30 GB/s behind the 4 MB
            # output bursts (SDMA packet round-robin across queues).
            hs = []
            for t in range(N_BT):
                h = hpool.tile([P, NP * 2], f32)
                nc.gpsimd.dma_start(h[:], ls[t * P : (t + 1) * P, :])
                hs.append(h)

            for t in range(N_BT):
                h = hs[t]
                o = opool.tile([P, NFFT * 2], f32)

                hk = h[:].rearrange("p (k c) -> p k c", c=2)
                ov = o[:].rearrange("p (k r c) -> p k r c", r=SPACING, c=2)

                for r in range(SPACING):
                    tmp = tpool.tile([P, NSEG, 2], f32)
                    # tmp = gamma[r] * H[:, k+1, :]           (ScalarE)
                    nc.scalar.mul(tmp[:], hk[:, 1:NP, :], ct[:, 8 + r : 9 + r])
                    # out[:, k, r, :] = alpha[r]*H[:, k, :] + tmp   (DVE, fused)
                    nc.vector.scalar_tensor_tensor(
                        ov[:, 0:NSEG, r, :],
                        hk[:, 0:NSEG, :],
                        ct[:, r : r + 1],
                        tmp[:],
                        mult,
                        add,
                    )

                # Last 8 subcarriers: coeffs vary along r, so use broadcast
                # reads of H[510]/H[511] against per-element coef tiles.
                h510 = h[:, 2 * (NP - 2) : 2 * (NP - 1)].unsqueeze(1).broadcast_to((P, 8, 2))
                h511 = h[:, 2 * (NP - 1) : 2 * NP].unsqueeze(1).broadcast_to((P, 8, 2))
                a_last = ct[:, 16:32].rearrange("p (r c) -> p r c", c=2)
                c_last = ct[:, 32:48].rearrange("p (r c) -> p r c", c=2)
                tl = lpool.tile([P, 8, 2], f32)
                nc.vector.tensor_mul(tl[:], h510, a_last)
                t2 = lpool.tile([P, 8, 2], f32)
                nc.vector.tensor_mul(t2[:], h511, c_last)
                o_last = o[:, NSEG * 16 : NFFT * 2].rearrange("p (r c) -> p r c", c=2)
                nc.vector.tensor_add(o_last, tl[:], t2[:])

                nc.sync.dma_start(out[t * P : (t + 1) * P, :], o[:])
    nc.compile()
    return nc


def _coef_tile(decay_param: np.ndarray) -> np.ndarray:
    """[128, 48] f32: cols 0:8 alpha[r], 8:16 gamma[r], 16:32 last-chunk
    coeff on H[510] (r,c-flattened), 32:48 last-chunk coeff on H[511]."""
    x = np.float32(np.asarray(decay_param).reshape(-1)[0])
    d = np.logaddexp(np.float32(0.0), x, dtype=np.float32)  # softplus
    r = np.arange(SPACING, dtype=np.float32)
    eps = np.float32(1e-12)
    # regular segments: x1 - x0 = 8
    wl = np.exp(-d * r, dtype=np.float32)
    wr = np.exp(-d * (np.float32(SPACING) - r), dtype=np.float32)
    w = wl + wr + eps
    alpha, gamma = wl / w, wr / w
    # last chunk: i = 4088 + r, x0 = 4088, x1 = 4095 (gap of 7);
    # y1 = hN = (15/8) H[511] - (7/8) H[510]
    wl2 = np.exp(-d * r, dtype=np.float32)
    wr2 = np.exp(-d * (np.float32(7.0) - r), dtype=np.float32)
    w2 = wl2 + wr2 + eps
    c511 = (wl2 + np.float32(1.875) * wr2) / w2
    c510 = -np.float32(0.875) * wr2 / w2
    row = np.concatenate(
        [alpha, gamma, np.repeat(c510, 2), np.repeat(c511, 2)]
    ).astype(np.float32)
    return np.broadcast_to(row, (P, 48)).copy()


def kernel(LS_ri, pilot_pos=None, decay_param=None, Nfft=None, **_unused):
    global _PROGRAM
    LS_ri = np.ascontiguousarray(np.asarray(LS_ri, dtype=np.float32))
    coef = _coef_tile(decay_param)

    if _PROGRAM is None:
        _PROGRAM = _build_program()
    nc = _PROGRAM

    in_maps = []
    for c in range(N_CORES):
        shard = LS_ri[c * B_LOC : (c + 1) * B_LOC].reshape(B_LOC, NP * 2)
        in_maps.append({"ls": shard, "coef": coef})

    res = run_bass_kernel_spmd(nc, in_maps, list(range(N_CORES))).results
    out = np.concatenate(
        [res[c]["out"].reshape(B_LOC, NFFT, 2) for c in range(N_CORES)], axis=0
    )
    return out


# revision 7
# speedup vs baseline: 1.0065x; 1.0065x over previous
"""Distance-aware comb-pilot interpolator for Trainium2 (8 NeuronCores).

Math: out[b, i, c] = (w_l[i] * H[b, j0(i), c] + w_r[i] * H[b, j1(i), c]) / w[i]
with pilots on the comb loc[k] = 8k (k = 0..511), Nfft = 4096.
For i = 8k + r (k < 511): j0 = k, j1 = k + 1 and the normalized weights
depend only on r:  alpha[r] = w_l/w, gamma[r] = w_r/w.
For the last 8 subcarriers (i = 4088..4095) the reference extrapolates a
virtual pilot hN = (15/8)H[511] - (7/8)H[510] at subcarrier 4095; folding it
in gives per-r coefficients on H[510] and H[511] directly.

All coefficients depend only on decay = softplus(decay_param) and are O(8)
host work; they ship to the device as one tiny [128, 48] constant tile.

Device kernel (per core, batch-sharded 512 rows): partition dim = batch.
Per 128-batch tile: ScalarE computes tmp = gamma[r] * H[:, k+1, :], then the
DVE fused op scalar_tensor_tensor writes out[:, k, r, :] =
(H[:, k, :] * alpha[r]) + tmp for all k in one pass.  The kernel moves
2.1 MB in / 16.8 MB out per core and is HBM-bound (~53 us roofline).
"""

import sys

import numpy as np

for _p in ("/opt/trn_rl_repo", "/root/.axon_site/_ro/trn_rl_repo"):
    if _p not in sys.path:
        sys.path.append(_p)

import concourse.bass as bass
import concourse.tile as tile
from concourse import bacc, mybir
from concourse.bass_utils import run_bass_kernel_spmd

N_CORES = 8
B, NP, NFFT, SPACING = 4096, 512, 4096, 8
B_LOC = B // N_CORES  # batch rows per core
NSEG = NP - 1  # regular 8-wide segments (k = 0..510)
P = 128  # SBUF partitions
N_BT = B_LOC // P  # 128-batch tiles per core

_PROGRAM = None


def _build_program():
    """One Bass program, identical on all cores (pure data parallel)."""
    nc = bacc.Bacc("TRN2", target_bir_lowering=False, debug=False)
    f32 = mybir.dt.float32
    ls = nc.dram_tensor("ls", [B_LOC, NP * 2], f32, kind="ExternalInput").ap()
    coef = nc.dram_tensor("coef", [P, 48], f32, kind="ExternalInput").ap()
    out = nc.dram_tensor("out", [B_LOC, NFFT * 2], f32, kind="ExternalOutput").ap()

    mult, add = mybir.AluOpType.mult, mybir.AluOpType.add

    # Per-r engine assignment (GpSimd rejected by walrus ISA check for
    # TensorScalarPtr, so only ScalarE muls + DVE fused ops).
    mul_eng = {r: "act" for r in range(SPACING)}
    stt_eng = {r: "dve" for r in range(SPACING)}
    KHALF = 256  # output ships in two 2.1 MB half-tiles per 128-batch tile

    with tile.TileContext(nc) as tc:
        with (
            tc.tile_pool(name="cpool", bufs=1) as cpool,
            tc.tile_pool(name="hpool", bufs=4) as hpool,
            tc.tile_pool(name="opool", bufs=3) as opool,
            tc.tile_pool(name="tpool", bufs=10) as tpool,
            tc.tile_pool(name="lpool", bufs=2) as lpool,
        ):
            # Preload every input before any output traffic exists — loads
            # issued mid-kernel crawl behind the output bursts (SDMA packet
            # round-robin across queues). HWDGE (sync) starts earliest.
            ct = cpool.tile([P, 48], f32)
            nc.sync.dma_start(ct[:], coef)
            hs = []
            for t in range(N_BT):
                h = hpool.tile([P, NP * 2], f32)
                nc.sync.dma_start(h[:], ls[t * P : (t + 1) * P, :])
                hs.append(h)

            for t in range(N_BT):
                h = hs[t]
                o = opool.tile([P, NFFT * 2], f32)

                hk = h[:].rearrange("p (k c) -> p k c", c=2)
                ov = o[:].rearrange("p (k r c) -> p k r c", r=SPACING, c=2)

                for half in range(2):
                    k0 = half * KHALF
                    nk = KHALF if half == 0 else NSEG - KHALF  # 256 | 255
                    for r in range(SPACING):
                        tmp = tpool.tile([P, KHALF, 2], f32)
                        tv = tmp[:, 0:nk, :]
                        # tmp = gamma[r] * H[:, k+1, :]
                        geng = nc.gpsimd if mul_eng[r] == "gp" else nc.scalar
                        if mul_eng[r] == "gp":
                            nc.gpsimd.tensor_scalar_mul(
                                tv, hk[:, k0 + 1 : k0 + nk + 1, :], ct[:, 8 + r : 9 + r]
                            )
                        else:
                            nc.scalar.mul(
                                tv, hk[:, k0 + 1 : k0 + nk + 1, :], ct[:, 8 + r : 9 + r]
                            )
                        # out[:, k, r, :] = alpha[r]*H[:, k, :] + tmp  (fused)
                        seng = nc.gpsimd if stt_eng[r] == "gp" else nc.vector
                        seng.scalar_tensor_tensor(
                            ov[:, k0 : k0 + nk, r, :],
                            hk[:, k0 : k0 + nk, :],
                            ct[:, r : r + 1],
                            tv,
                            mult,
                            add,
                        )

                    if half == 1:
                        # Last 8 subcarriers: coeffs vary along r — broadcast
                        # H[510]/H[511] against per-element coef tiles.
                        h510 = h[:, 2 * (NP - 2) : 2 * (NP - 1)].unsqueeze(1).broadcast_to((P, 8, 2))
                        h511 = h[:, 2 * (NP - 1) : 2 * NP].unsqueeze(1).broadcast_to((P, 8, 2))
                        a_last = ct[:, 16:32].rearrange("p (r c) -> p r c", c=2)
                        c_last = ct[:, 32:48].rearrange("p (r c) -> p r c", c=2)
                        tl = lpool.tile([P, 8, 2], f32)
                        nc.vector.tensor_mul(tl[:], h510, a_last)
                        t2 = lpool.tile([P, 8, 2], f32)
                        nc.vector.tensor_mul(t2[:], h511, c_last)
                        o_last = o[:, NSEG * 16 : NFFT * 2].rearrange("p (r c) -> p r c", c=2)
                        nc.vector.tensor_add(o_last, tl[:], t2[:])

                    lo, hi = half * NFFT, (half + 1) * NFFT
                    nc.sync.dma_start(
                        out[t * P : (t + 1) * P, lo:hi], o[:, lo:hi]
                    )
    nc.compile()
    return nc


def _coef_tile(decay_param: np.ndarray) -> np.ndarray:
    """[128, 48] f32: cols 0:8 alpha[r], 8:16 gamma[r], 16:32 last-chunk
    coeff on H[510] (r,c-flattened), 32:48 last-chunk coeff on H[511]."""
    x = np.float32(np.asarray(decay_param).reshape(-1)[0])
    d = np.logaddexp(np.float32(0.0), x, dtype=np.float32)  # softplus
    r = np.arange(SPACING, dtype=np.float32)
    eps = np.float32(1e-12)
    # regular segments: x1 - x0 = 8
    wl = np.exp(-d * r, dtype=np.float32)
    wr = np.exp(-d * (np.float32(SPACING) - r), dtype=np.float32)
    w = wl + wr + eps
    alpha, gamma = wl / w, wr / w
    # last chunk: i = 4088 + r, x0 = 4088, x1 = 4095 (gap of 7);
    # y1 = hN = (15/8) H[511] - (7/8) H[510]
    wl2 = np.exp(-d * r, dtype=np.float32)
    wr2 = np.exp(-d * (np.float32(7.0) - r), dtype=np.float32)
    w2 = wl2 + wr2 + eps
    c511 = (wl2 + np.float32(1.875) * wr2) / w2
    c510 = -np.float32(0.875) * wr2 / w2
    row = np.concatenate(
        [alpha, gamma, np.repeat(c510, 2), np.repeat(c511, 2)]
    ).astype(np.float32)
    return np.broadcast_to(row, (P, 48)).copy()


def kernel(LS_ri, pilot_pos=None, decay_param=None, Nfft=None, **_unused):
    global _PROGRAM
    LS_ri = np.ascontiguousarray(np.asarray(LS_ri, dtype=np.float32))
    coef = _coef_tile(decay_param)

    if _PROGRAM is None:
        _PROGRAM = _build_program()
    nc = _PROGRAM

    in_maps = []
    for c in range(N_CORES):
        shard = LS_ri[c * B_LOC : (c + 1) * B_LOC].reshape(B_LOC, NP * 2)
        in_maps.append({"ls": shard, "coef": coef})

    res = run_bass_kernel_spmd(nc, in_maps, list(range(N_CORES))).results
    out = np.concatenate(
        [res[c]["out"].reshape(B_LOC, NFFT, 2) for c in range(N_CORES)], axis=0
    )
    return out
